# revision 25
# baseline (speedup 1.0000x reference)
"""Multi-head attention (B=2, S=2048, D=1024, H=16) on 8 Trainium2 cores.

Sharding: pure tensor-parallel over heads (2 heads/core). Each core
computes QKV + attention for its 2 heads over the full sequence, then a
PARTIAL output projection (contraction over its 128 head-dims) for the
full [B, S, D] output. The host sums the 8 partial outputs — no
collectives at all on device.

Device pipeline per core (SPMD, identical program, shard-specific data):
  - PE warmup matmuls while x^T streams in (HAM clock ramp)
  - K(0) full + Q(0, qt0) projection, then attention starts; remaining
    Q/V projections and later the out-projection pieces are interleaved
    into the attention stream as PE fillers (the attention loop is
    paced by the ACT engine's exp throughput, leaving PE slack)
  - scores: head A (PE rows 0-63) and head B (rows 64-127) matmuls are
    emitted back-to-back -> run concurrently via row tiling
  - exp on ACT (PSUM [128,1024] per head), A*V with a ones column on V
    producing the softmax denominator (M=65)
  - normalize: reciprocal on the [1,512] denom row, DRAM bounce for the
    partition-broadcast, DVE mul assembling attn^T [128 dims, 512 q]
  - out-projection: lhsT = attn^T slice [128, 128], rhs = w_out core
    slice [128, 512]; bias comes in as data (zeros on cores 1-7)

PSUM budget (8 banks): sA [128,1024] = 2, sB [128,1024] = 2,
av [65,512] x3 = 3, o [128,512] x1 = 1. QKV proj / V / warmup /
outproj borrow the "o" slot.
"""
import os
import sys

sys.path.insert(0, "/opt/trn_rl_repo")

import numpy as np
import ml_dtypes

import concourse.bass as bass
import concourse.tile as tile
from concourse import bacc, mybir
from concourse import bass_utils

B = 2
S = 2048
D = 1024
H = 16
DH = 64
N_CORES = 8
HEADS_PER_CORE = H // N_CORES          # 2
N_CH = D // 128                        # 8 contraction chunks
N_QT = S // 512                        # 4 q tiles
N_KC = S // 128                        # 16 k chunks

F32 = mybir.dt.float32
BF16 = mybir.dt.bfloat16
FP8 = mybir.dt.float8e4

_compiled = None
last_results = None


def _build():
    nc = bacc.Bacc(
        "TRN2",
        target_bir_lowering=False,
        debug=False,
        enable_asserts=True,
        num_devices=N_CORES,
    )

    xtb = nc.dram_tensor("xtb", [B, 128, N_CH, S], BF16, kind="ExternalInput").ap()
    wqt = nc.dram_tensor("wqt", [128, N_CH, 128], BF16, kind="ExternalInput").ap()
    wkt = nc.dram_tensor("wkt", [128, N_CH, 128], BF16, kind="ExternalInput").ap()
    wvt = nc.dram_tensor("wvt", [128, N_CH, 128], BF16, kind="ExternalInput").ap()
    wot = nc.dram_tensor("wot", [128, D], BF16, kind="ExternalInput").ap()
    oc = nc.dram_tensor("oc", [B, S, D], BF16, kind="ExternalOutput").ap()

    EXP = mybir.ActivationFunctionType.Exp
    SCALE = DH ** -0.5

    with tile.TileContext(nc) as tc:
        with (
            tc.tile_pool(name="w", bufs=1) as wp,
            tc.tile_pool(name="qkt", bufs=1) as qktp,
            tc.tile_pool(name="vsb", bufs=1) as vsbp,
            tc.tile_pool(name="xtb", bufs=2) as xtbp,
            tc.tile_pool(name="pt", bufs=4) as ptp,
            tc.tile_pool(name="at", bufs=3) as atp,
            tc.tile_pool(name="norm", bufs=4) as normp,
            tc.tile_pool(name="outsb", bufs=2) as outp,
            tc.tile_pool(name="dramsc", bufs=4, space="DRAM") as dramsc,
            tc.tile_pool(name="sps", bufs=2, space="PSUM") as sps,
            tc.tile_pool(name="avps", bufs=2, space="PSUM") as avps,
            tc.tile_pool(name="ops", bufs=2, space="PSUM") as ops,
        ):
            # ---- weights ----
            wqt_sb = wp.tile([128, N_CH * 128], BF16)
            nc.sync.dma_start(wqt_sb[:], wqt[:].rearrange("p c e -> p (c e)"))
            wkt_sb = wp.tile([128, N_CH * 128], BF16)
            nc.sync.dma_start(wkt_sb[:], wkt[:].rearrange("p c e -> p (c e)"))
            wvt_sb = wp.tile([128, N_CH * 128], BF16)
            nc.sync.dma_start(wvt_sb[:], wvt[:].rearrange("p c e -> p (c e)"))
            wot_sb = wp.tile([128, D], BF16)
            nc.sync.dma_start(wot_sb[:], wot[:])

            # ---- PE warmup while DMAs stream ----
            warm = wp.tile([128, 512], BF16)
            nc.gpsimd.memset(warm[:], 0.0)
            for i in range(16):
                wps = ops.tile([128, 512], F32, tag="o", name="wps")
                nc.tensor.matmul(wps[:], lhsT=warm[:, 0:128], rhs=warm[:],
                                 start=True, stop=True)

            # pre-load the exp activation table set during the DMA wait
            # (first call to a new set costs ~2.7us)
            twarm = wp.tile([1, 16], F32)
            nc.scalar.activation(twarm[:], warm[0:1, 0:16], EXP)

            xtb_sbs = [None, None]
            Qt, Kt, Vs = [], [], []
            for b in range(B):
                Qt.append(qktp.tile([128, S], BF16, tag=f"qt{b}", name=f"qt{b}"))
                Kt.append(qktp.tile([128, S], BF16, tag=f"kt{b}", name=f"kt{b}"))
                Vs.append(vsbp.tile([128, N_KC, 2, 128], BF16, tag=f"v{b}",
                                    name=f"v{b}"))

            def emit_xtb_load(b):
                t_ = xtbp.tile([128, N_CH * S], BF16, tag="xtb", name="xtb_sb")
                xtb_sbs[b] = t_
                for ch in range(N_CH):
                    nc.sync.dma_start(t_[:, ch * S:(ch + 1) * S], xtb[b, :, ch, :])

            def emit_qk_pair(b, t):
                """Q^T and K^T slice t, 8 accumulating matmuls each."""
                for w_sb, dst in ((wqt_sb, Qt[b]), (wkt_sb, Kt[b])):
                    ps_ = ops.tile([128, 512], F32, tag="o", name="qk_ps")
                    for ch in range(N_CH):
                        nc.tensor.matmul(
                            ps_[:],
                            lhsT=w_sb[:, ch * 128:(ch + 1) * 128],
                            rhs=xtb_sbs[b][:, ch * S + t * 512:
                                           ch * S + (t + 1) * 512],
                            start=(ch == 0), stop=(ch == N_CH - 1),
                        )
                    nc.vector.tensor_copy(dst[:, t * 512:(t + 1) * 512], ps_[:])

            def emit_v(b, sts):
                v_sb = Vs[b]
                for st in sts:
                    v_ps = ops.tile([128, 512], F32, tag="o", name="v_ps")
                    for ch in range(N_CH):
                        nc.tensor.matmul(
                            v_ps[:, 0:128],
                            lhsT=xtb_sbs[b][:, ch * S + st * 128:
                                            ch * S + (st + 1) * 128],
                            rhs=wvt_sb[:, ch * 128:(ch + 1) * 128],
                            start=(ch == 0), stop=(ch == N_CH - 1),
                        )
                    nc.vector.tensor_copy(
                        v_sb[:, st, :, 0:64],
                        v_ps[:, 0:128].rearrange("p (h e) -> p h e", e=64)
                    )

            # ---------- filler machinery ----------
            # queue of (cost_ns, thunk); injected into attention PE slack
            filler_q = []

            def inject_fillers(budget_ns):
                while filler_q and budget_ns > 0:
                    cost, thunk = filler_q.pop(0)
                    with tc.high_priority(offset=-10_000_000):
                        thunk()
                    budget_ns -= cost

            def drain_fillers():
                n = 0
                while filler_q:
                    cost, thunk = filler_q.pop(0)
                    thunk(use_scalar=(n % 2 == 1)) if getattr(
                        thunk, "is_piece", False) else thunk()
                    n += 1

            # ---------- attention ----------
            def emit_outproj_piece(b, qt, si, et, use_scalar=False):
                """out[b, qt*512+si*128 : +128, et*512 : +512] partial."""
                at_sb = attnT_tiles[(b, qt)]
                o_ps = ops.tile([128, 512], F32, tag="o", name="o_ps")
                nc.tensor.matmul(
                    o_ps[:],
                    lhsT=at_sb[:, si * 128:(si + 1) * 128],
                    rhs=wot_sb[:, et * 512:(et + 1) * 512],
                    start=True, stop=True,
                )
                out_sb = outp.tile([128, 512], BF16, tag="osb", name="out_sb")
                if use_scalar:
                    nc.scalar.copy(out_sb[:], o_ps[:])
                else:
                    nc.vector.tensor_copy(out_sb[:], o_ps[:])
                s0 = qt * 512 + si * 128
                nc.sync.dma_start(
                    oc[b, s0:s0 + 128, et * 512:(et + 1) * 512], out_sb[:])

            attnT_tiles = {}

            def att_unit(b, qt, startup_v=False):
                """Both heads' attention for q-slice qt of batch b."""
                qs = slice(qt * 512, (qt + 1) * 512)
                avA = avps.tile([65, 512], F32, tag="av", name="avA")
                avB = avps.tile([65, 512], F32, tag="av", name="avB")
                p2 = None
                for c in range(N_KC):
                    # scores for chunk c, both heads: A -> cols 0:512 (bank
                    # 1), B -> cols 512:1024 (bank 2); adjacent emission ->
                    # the two matmuls run concurrently via PE row tiling
                    s_ = sps.tile([128, 1024], F32, tag="s", name="s_")
                    ks = slice(c * 128, (c + 1) * 128)
                    nc.tensor.matmul(
                        s_[:, 0:512],
                        lhsT=Kt[b][0:64, ks], rhs=Qt[b][0:64, qs],
                        start=True, stop=True,
                    )
                    nc.tensor.matmul(
                        s_[:, 512:1024],
                        lhsT=Kt[b][64:128, ks], rhs=Qt[b][64:128, qs],
                        start=True, stop=True,
                    )
                    p_ = ptp.tile([128, 1024], BF16, tag="p", name="p_")
                    nc.scalar.activation(p_[:], s_[:], EXP, scale=SCALE)
                    if startup_v and c % 2 == 0 and c < 14:
                        emit_v(b, (c + 2, c + 3))
                        if c % 4 == 0 and c // 4 + 1 < N_QT:
                            emit_qk_pair(b, c // 4 + 1)
                    else:
                        inject_fillers(550)
                    for hv, av in ((0, avA), (1, avB)):
                        nc.tensor.matmul(
                            av[:],
                            lhsT=Vs[b][:, c, hv, 0:65],
                            rhs=p_[:, hv * 512:(hv + 1) * 512],
                            start=(c == 0), stop=(c == N_KC - 1),
                            skip_group_check=True,
                        )
                # normalize: copy av to SBUF fast (frees the psum bank),
                # then broadcast the raw denominator and reciprocal on 64
                # lanes -- the chain runs off the critical path
                at_sb = atp.tile([128, 512], BF16, tag="at", name="at_sb")
                attnT_tiles[(b, qt)] = at_sb
                for h, av in ((0, avA), (1, avB)):
                    av_sb = normp.tile([65, 512], F32, tag="avsb", name="av_sb")
                    nc.vector.tensor_copy(av_sb[:], av[0:65, :])
                    eng = nc.sync if h == 0 else nc.gpsimd
                    den_d = dramsc.tile([512], F32, tag="dend", name="den_d")
                    eng.dma_start(
                        den_d[:].rearrange("(a q) -> a q", a=1),
                        av_sb[64:65, :])
                    den_bc = normp.tile([64, 512], F32, tag="dbc", name="den_bc")
                    eng.dma_start(
                        den_bc[:],
                        den_d[:].rearrange("(a q) -> a q", a=1)
                        .broadcast_to([64, 512]),
                    )
                    rec_bc = normp.tile([64, 512], F32, tag="rbc", name="rec_bc")
                    nc.vector.reciprocal(rec_bc[:], den_bc[:])
                    nc.vector.tensor_mul(
                        at_sb[h * 64:(h + 1) * 64, :], av_sb[0:64, :],
                        rec_bc[:])
                # out-projection pieces go out right here: they execute
                # in the qt-boundary ACT bubble (gated on the normalize
                # chain). The final unit's pieces use both ACT and DVE for
                # the psum->sbuf cast since exp is done by then.
                last = (b == 1 and qt == N_QT - 1)
                for si in range(4):
                    for et in range(2):
                        emit_outproj_piece(b, qt, si, et,
                                           use_scalar=(last and si % 2 == 1))

            # ================= pipeline =================
            emit_xtb_load(0)
            nc.gpsimd.memset(Vs[0][:, :, :, 64:65], 1.0)
            nc.gpsimd.memset(Vs[1][:, :, :, 64:65], 1.0)

            # Q/K slice 0 + V(0, c0/c1); the rest streams in as startup
            # fillers inside att_unit(0, 0)
            emit_qk_pair(0, 0)
            emit_v(0, (0, 1))
            emit_xtb_load(1)

            # batch-1 projections enter the filler queue (QK first)
            for t in range(N_QT):
                filler_q.append((3400, (lambda t_=t: emit_qk_pair(1, t_))))
            for st in range(N_KC):
                filler_q.append((700, (lambda st_=st: emit_v(1, (st_,)))))

            att_unit(0, 0, startup_v=True)
            for qt in range(1, N_QT):
                att_unit(0, qt)
            for qt in range(N_QT):
                att_unit(1, qt)
            drain_fillers()

    nc.compile()
    return nc


def _prep_chunked(a_t):
    """[Din, E] (already transposed) -> [128, Din//128, E] SBUF-chunk layout."""
    din, e = a_t.shape
    return np.ascontiguousarray(
        a_t.reshape(din // 128, 128, e).transpose(1, 0, 2)
    )


def kernel(x, w_qkv, w_out, b_out):
    global _compiled, last_results
    if _compiled is None:
        _compiled = _build()
    nc = _compiled

    x = np.asarray(x, dtype=np.float32)
    w_qkv = np.asarray(w_qkv, dtype=np.float32)
    w_out = np.asarray(w_out, dtype=np.float32)
    b_out = np.asarray(b_out, dtype=np.float32)

    # x^T in chunk layout: [B, 128, N_CH, S], bf16
    xt_full = x.transpose(0, 2, 1)  # [B, D, S]
    xtb_prep = np.ascontiguousarray(
        xt_full.reshape(B, N_CH, 128, S).transpose(0, 2, 1, 3)
    ).astype(ml_dtypes.bfloat16)

    in_maps = []
    for c in range(N_CORES):
        hA, hB = HEADS_PER_CORE * c, HEADS_PER_CORE * c + 1
        rows = np.r_[hA * DH:(hA + 1) * DH, hB * DH:(hB + 1) * DH]
        wq = w_qkv[rows, :]               # [128, D]
        wk = w_qkv[D + rows, :]
        wv = w_qkv[2 * D + rows, :]
        # partial out-projection: contraction over this core's 128 dims
        wo_core = np.ascontiguousarray(w_out[:, rows].T)   # [128, D]
        in_maps.append({
            "xtb": xtb_prep,
            "wqt": _prep_chunked(np.ascontiguousarray(wq.T)).astype(ml_dtypes.bfloat16),
            "wkt": _prep_chunked(np.ascontiguousarray(wk.T)).astype(ml_dtypes.bfloat16),
            "wvt": _prep_chunked(np.ascontiguousarray(wv.T)).astype(ml_dtypes.bfloat16),
            "wot": wo_core.astype(ml_dtypes.bfloat16),
        })

    last_results = bass_utils.run_bass_kernel_spmd(
        nc, in_maps, core_ids=list(range(N_CORES))
    )
    out = last_results.results[0]["oc"].astype(np.float64)
    for c in range(1, N_CORES):
        out += last_results.results[c]["oc"]
    out += b_out
    return out.astype(np.float32)
